# revision 1
# baseline (speedup 1.0000x reference)
"""CQT extractor kernel for Trainium2 (8 NeuronCores, data-parallel over batch).

Pipeline per core (2 audio rows):
  STFT-as-matmul with Hermitian folding (1024-long contraction instead of
  2048), magnitude via ACT Square/Sqrt, CQT projection matmul, log10.

Host side does only data movement (reflect pad, chunk-reversed copy for the
fold) and constant table generation; all FLOPs run on device.
"""

import math
from contextlib import ExitStack

import numpy as np


import concourse.tile as tile
from concourse import bacc, mybir
from concourse.bass_utils import run_bass_kernel_spmd
from concourse.masks import make_identity

# ---- problem constants (hardcoded per contest rules) ----
B = 16
L = 1310720
SR = 22050
HOP = 512
NFFT = 2048
NBINS = 84
BPO = 12
FMIN = 27.5

NF = 1 + L // HOP            # 2561 frames
PAD = NFFT // 2              # 1024
LP = L + 2 * PAD             # 1312768 reflect-padded length

NCORES = 8
ROWS_PER_CORE = B // NCORES  # 2

# frame tiling: 6 uniform tiles of 428 frames (fp32r needs even moving dim);
# frames past NF-1 are computed on zero padding and never written out
T_SIZES = [428] * 6
T_STARTS = [428 * i for i in range(6)]
T_ALLOC = 428

NGRP = 14                    # 128-chunk transpose groups per frame tile
WCH = NGRP * 128             # 1792 chunks staged per frame tile
NCH_PAD = 4 * T_STARTS[-1] + WCH + 1   # chunks incl. zero pad (+1 for +1 shift)
PADLEN = 128 * NCH_PAD

F32 = mybir.dt.float32
F32R = mybir.dt.float32r
LOG10E = 1.0 / math.log(10.0)


def _host_tables():
    """Folded DFT matrices and CQT weights, float64 -> float32."""
    j = np.arange(1024)
    n = (j + 1).astype(np.float64)          # contraction index j <-> sample n=j+1
    win = 0.5 * (1.0 - np.cos(2.0 * np.pi * n / NFFT))
    ang = 2.0 * np.pi * np.outer(n, np.arange(1024, dtype=np.float64)) / NFFT
    wc = win[:, None] * np.cos(ang)
    ws = win[:, None] * np.sin(ang)
    wc[1023] *= 0.5           # n=1024 term is double-counted by the fold
    ws[1023] = 0.0
    sf = np.fft.rfftfreq(NFFT, 1.0 / SR)[:1024]
    cf = FMIN * 2.0 ** (np.arange(NBINS, dtype=np.float64) / BPO)
    wq = np.exp(-np.abs(sf[:, None] - cf[None, :]) / (0.1 * cf[None, :]))
    return (
        np.ascontiguousarray(wc, dtype=np.float32),
        np.ascontiguousarray(ws, dtype=np.float32),
        np.ascontiguousarray(wq, dtype=np.float32),
    )


def _build_program():
    nc = bacc.Bacc("TRN2", target_bir_lowering=False, debug=False,
                   num_devices=NCORES)
    xp = nc.dram_tensor("xp", [ROWS_PER_CORE, PADLEN], F32R,
                        kind="ExternalInput").ap()
    zp = nc.dram_tensor("zp", [ROWS_PER_CORE, PADLEN], F32R,
                        kind="ExternalInput").ap()
    wc = nc.dram_tensor("wc", [8, 8, 128, 128], F32R, kind="ExternalInput").ap()
    ws = nc.dram_tensor("ws", [8, 8, 128, 128], F32R, kind="ExternalInput").ap()
    wq = nc.dram_tensor("wq", [1024, NBINS], F32R, kind="ExternalInput").ap()
    out = nc.dram_tensor("out", [ROWS_PER_CORE, NBINS, NF], F32,
                         kind="ExternalOutput").ap()

    with tile.TileContext(nc) as tc:
        with ExitStack() as ctx:
            _emit(ctx, tc, xp, zp, wc, ws, wq, out)
    nc.compile()
    return nc


def _emit(ctx, tc, xp, zp, wc, ws, wq, out):
    nc = tc.nc
    SQ = mybir.ActivationFunctionType.Square
    SQRT = mybir.ActivationFunctionType.Sqrt
    LN = mybir.ActivationFunctionType.Ln

    consts = ctx.enter_context(tc.tile_pool(name="consts", bufs=1))
    natp = ctx.enter_context(tc.tile_pool(name="natp", bufs=6))
    stage = ctx.enter_context(tc.tile_pool(name="stage", bufs=2))
    eo = ctx.enter_context(tc.tile_pool(name="eo", bufs=2))
    magp = ctx.enter_context(tc.tile_pool(name="magp", bufs=2))
    sqp = ctx.enter_context(tc.tile_pool(name="sqp", bufs=3))
    outp = ctx.enter_context(tc.tile_pool(name="outp", bufs=2))
    ps_mm = ctx.enter_context(tc.tile_pool(name="ps_mm", bufs=5, space="PSUM"))
    ps_tp = ctx.enter_context(tc.tile_pool(name="ps_tp", bufs=2, space="PSUM"))
    ps_cq = ctx.enter_context(tc.tile_pool(name="ps_cq", bufs=1, space="PSUM"))

    # constants (staged f32 -> rounded f32r copies)
    # [p, i_colblock, a_ktile, f] so each 512KB W-block DMA is contiguous
    wc_sb = consts.tile([128, 8, 8, 128], F32R, tag="wc_sb")
    ws_sb = consts.tile([128, 8, 8, 128], F32R, tag="ws_sb")
    wq_sb = consts.tile([128, 8, NBINS], F32R, tag="wq_sb")
    # direct f32r DMA of host-preblocked W, one 512KB DMA per column block,
    # pair-0 weights land first
    for i in range(8):
        nc.gpsimd.dma_start(wc_sb[:, i], wc[i].rearrange("a p f -> p a f"))
        nc.scalar.dma_start(ws_sb[:, i], ws[i].rearrange("a p f -> p a f"))
    nc.sync.dma_start(wq_sb[:], wq.rearrange("(a p) k -> p a k", a=8))
    ident = consts.tile([128, 128], F32, tag="ident")
    make_identity(nc, ident[:])
    identr = consts.tile([128, 128], F32R, tag="identr")
    nc.vector.tensor_copy(identr[:], ident[:])
    lnbias = consts.tile([128, 1], F32, tag="lnbias")
    nc.gpsimd.memset(lnbias[:], 1e-10)

    stage_count = [0]

    def emit_stage(r, it):
        """DMA + PE transpose + copyback + fold adds for one frame tile."""
        # during startup the scalar queue carries the W sin tables; route the
        # first two tiles' z loads through sync instead
        zq = nc.sync if stage_count[0] < 2 else nc.scalar
        stage_count[0] += 1
        T = T_SIZES[it]
        f0 = T_STARTS[it]
        cbase = 4 * f0
        Q = WCH // 4
        dts = stage.tile([128, 4, Q], F32, tag="dts")
        rev = stage.tile([128, 4, Q], F32, tag="rev")
        for g in range(NGRP):
            off = (cbase + 128 * g) * 128
            natx = natp.tile([128, 128], F32R, tag="natx")
            nc.sync.dma_start(
                natx[:],
                xp[r, off + 1: off + 1 + 128 * 128].rearrange(
                    "(c s) -> c s", s=128),
            )
            tpx = ps_tp.tile([128, 128], F32R, tag="tp")
            nc.tensor.transpose(tpx[:], natx[:], identr[:])
            nc.vector.tensor_copy(dts[:, :, 32 * g: 32 * (g + 1)],
                                  tpx.rearrange("p (q a) -> p a q", a=4))

            natz = natp.tile([128, 128], F32R, tag="natz")
            zq.dma_start(
                natz[:],
                zp[r, off: off + 128 * 128].rearrange("(c s) -> c s", s=128),
            )
            tpz = ps_tp.tile([128, 128], F32R, tag="tp")
            nc.tensor.transpose(tpz[:], natz[:], identr[:])
            nc.vector.tensor_copy(rev[:, :, 32 * g: 32 * (g + 1)],
                                  tpz.rearrange("p (q a) -> p a q", a=4))

        # folded operands: E[j,t]=x[512t+j+1]+x[512t+2047-j], O = diff
        # E term chunk c=4t+a -> phase a%4, q=t+a//4 (contiguous reads);
        # partner chunk c=4t+15-a -> phase (15-a)%4, q=t+(15-a)//4
        e4 = eo.tile([128, 8, T_ALLOC], F32R, tag="e4")
        o4 = eo.tile([128, 8, T_ALLOC], F32R, tag="o4")
        for a in range(8):
            d_ap = dts[:, a % 4, a // 4: a // 4 + T]
            r_ap = rev[:, (15 - a) % 4, (15 - a) // 4: (15 - a) // 4 + T]
            nc.vector.tensor_add(e4[:, a, :T], d_ap, r_ap)
            nc.gpsimd.tensor_sub(o4[:, a, :T], d_ap, r_ap)
        return e4, o4

    def emit_dft(r, it, e4, o4):
        """DFT matmuls + magnitude for one frame tile."""
        T = T_SIZES[it]
        mag = magp.tile([128, 8, T_ALLOC], F32R, tag="mag")
        for i in range(8):
            ps_re = ps_mm.tile([128, T_ALLOC], F32, tag="mm")
            for a in range(8):
                nc.tensor.matmul(
                    ps_re[:, :T],
                    wc_sb[:, i, a],
                    e4[:, a, :T],
                    start=(a == 0), stop=(a == 7),
                )
            ps_im = ps_mm.tile([128, T_ALLOC], F32, tag="mm")
            for a in range(8):
                nc.tensor.matmul(
                    ps_im[:, :T],
                    ws_sb[:, i, a],
                    o4[:, a, :T],
                    start=(a == 0), stop=(a == 7),
                )
            sq = sqp.tile([128, T_ALLOC], F32, tag="sq")
            nc.scalar.activation(sq[:, :T], ps_re[:, :T], SQ)
            sq2 = sqp.tile([128, T_ALLOC], F32, tag="sq2")
            nc.scalar.activation(sq2[:, :T], ps_im[:, :T], SQ)
            nc.vector.tensor_add(sq[:, :T], sq[:, :T], sq2[:, :T])
            nc.scalar.activation(mag[:, i, :T], sq[:, :T], SQRT)
        return mag

    def emit_cqt(r, it, mag):
        """CQT projection, log10, store."""
        T = T_SIZES[it]
        f0 = T_STARTS[it]
        ps_c = ps_cq.tile([NBINS, T_ALLOC], F32, tag="ps_c")
        for i in range(8):
            nc.tensor.matmul(
                ps_c[:, :T],
                wq_sb[:, i, :],
                mag[:, i, :T],
                start=(i == 0), stop=(i == 7),
            )
        V = min(T, NF - f0)          # valid (non-garbage) frames
        outt = outp.tile([NBINS, T_ALLOC], F32, tag="outt")
        nc.scalar.activation(outt[:, :V], ps_c[:, :V], LN,
                             bias=lnbias[:NBINS])
        nc.vector.tensor_scalar_mul(outt[:, :V], outt[:, :V], LOG10E)
        nc.sync.dma_start(out[r, :, f0: f0 + V], outt[:, :V])

    # software pipeline: PE order per slot is [transposes k+1][cqt k-1][dft k]
    # so the magnitude drain of tile k-1 and fold adds of k+1 hide under PE work
    tiles = [(r, it) for r in range(ROWS_PER_CORE) for it in range(6)]
    staged = emit_stage(*tiles[0])
    pending = None          # (r, it, mag) awaiting cqt
    for k, (r, it) in enumerate(tiles):
        nxt = emit_stage(*tiles[k + 1]) if k + 1 < len(tiles) else None
        if pending is not None:
            emit_cqt(*pending)
        mag = emit_dft(r, it, *staged)
        pending = (r, it, mag)
        staged = nxt
    emit_cqt(*pending)


_PROGRAM_CACHE = {}


def _get_program():
    if "nc" not in _PROGRAM_CACHE:
        _PROGRAM_CACHE["nc"] = _build_program()
    return _PROGRAM_CACHE["nc"]


def kernel(audio):
    audio = np.asarray(audio, dtype=np.float32)
    assert audio.shape == (B, L), audio.shape

    # host data movement: reflect pad + zero pad + within-chunk-reversed copy
    xpad = np.zeros((B, PADLEN), dtype=np.float32)
    xpad[:, :LP] = np.pad(audio, ((0, 0), (PAD, PAD)), mode="reflect")
    z = np.ascontiguousarray(
        xpad.reshape(B, NCH_PAD, 128)[:, :, ::-1]).reshape(B, PADLEN)

    wc, ws, wq = _host_tables()
    # (8_i, 8_a, 128_p, 128_f) blocks: wcb[i,a,p,f] = wc[128a+p, 128i+f]
    wc = np.ascontiguousarray(
        wc.reshape(8, 128, 8, 128).transpose(2, 0, 1, 3))
    ws = np.ascontiguousarray(
        ws.reshape(8, 128, 8, 128).transpose(2, 0, 1, 3))
    nc = _get_program()

    in_maps = []
    for c in range(NCORES):
        rows = slice(ROWS_PER_CORE * c, ROWS_PER_CORE * (c + 1))
        in_maps.append({
            "xp": np.ascontiguousarray(xpad[rows]),
            "zp": np.ascontiguousarray(z[rows]),
            "wc": wc, "ws": ws, "wq": wq,
        })

    res = run_bass_kernel_spmd(nc, in_maps, core_ids=list(range(NCORES)))
    out = np.concatenate([res.results[c]["out"] for c in range(NCORES)], axis=0)
    return np.ascontiguousarray(out, dtype=np.float32)



# revision 5
# speedup vs baseline: 1.5287x; 1.5287x over previous
"""CQT extractor kernel for Trainium2 (8 NeuronCores, data-parallel over batch).

Pipeline per core (2 audio rows), all fp16 on the hot path:
  STFT-as-matmul with Hermitian folding (1024-long contraction), output
  truncated to the lowest 512 of 1025 rfft bins (CQT weights above ~5.5 kHz
  are < 2e-3 and contribute ~1e-4 relative error), magnitude via ACT
  Square/Sqrt split across scalar/vector/gpsimd, CQT projection matmul, log10.

The PE transposes use a permuted identity that de-interleaves the 4
chunks-per-hop phase structure, so every later access is unit-stride and the
PSUM->SBUF copies qualify for the DVE 2-byte fast path.
"""

import math
from contextlib import ExitStack

import numpy as np

import concourse.tile as tile
from concourse import bacc, mybir
from concourse.bass_utils import run_bass_kernel_spmd

# ---- problem constants (hardcoded per contest rules) ----
B = 16
L = 1310720
SR = 22050
HOP = 512
NFFT = 2048
NBINS = 84
BPO = 12
FMIN = 27.5

NF = 1 + L // HOP            # 2561 frames
PAD = NFFT // 2              # 1024
LP = L + 2 * PAD             # 1312768 reflect-padded length

NCORES = 8
ROWS_PER_CORE = B // NCORES  # 2

NFREQ = 512                  # truncated rfft bins (of 1025)
NI = NFREQ // 128            # 4 freq blocks

# frame tiling: 6 uniform tiles of 428 frames; frames past NF-1 are computed
# on zero padding and never written out
T_SIZES = [428] * 6
T_STARTS = [428 * i for i in range(6)]
T_ALLOC = 428

NGRP = 14                    # 128-chunk transpose groups per frame tile
WCH = NGRP * 128             # 1792 chunks staged per frame tile
Q = WCH // 4                 # 448 per-phase chunk columns
NCH_PAD = 4 * T_STARTS[-1] + WCH + 1   # chunks incl. zero pad (+1 for +1 shift)
PADLEN = 128 * NCH_PAD

F32 = mybir.dt.float32
F16 = mybir.dt.float16
LOG10E = 1.0 / math.log(10.0)


def _host_tables():
    """Folded DFT matrices (512 bins) and CQT weights, float64 -> float16."""
    j = np.arange(1024)
    n = (j + 1).astype(np.float64)          # contraction index j <-> sample n=j+1
    win = 0.5 * (1.0 - np.cos(2.0 * np.pi * n / NFFT))
    ang = 2.0 * np.pi * np.outer(n, np.arange(NFREQ, dtype=np.float64)) / NFFT
    wc = win[:, None] * np.cos(ang)
    ws = win[:, None] * np.sin(ang)
    wc[1023] *= 0.5           # n=1024 term is double-counted by the fold
    ws[1023] = 0.0
    sf = np.fft.rfftfreq(NFFT, 1.0 / SR)[:NFREQ]
    cf = FMIN * 2.0 ** (np.arange(NBINS, dtype=np.float64) / BPO)
    wq = np.exp(-np.abs(sf[:, None] - cf[None, :]) / (0.1 * cf[None, :]))
    return (
        np.ascontiguousarray(wc, dtype=np.float16),
        np.ascontiguousarray(ws, dtype=np.float16),
        np.ascontiguousarray(wq, dtype=np.float16),
    )


def _perm_identity():
    """Permutation matrix: transpose output column a*32+q <- chunk row 4q+a."""
    p = np.zeros((128, 128), dtype=np.float16)
    jj = np.arange(128)
    p[4 * (jj % 32) + jj // 32, jj] = 1.0
    return p


def _build_program():
    nc = bacc.Bacc("TRN2", target_bir_lowering=False, debug=False,
                   num_devices=NCORES)
    xp = nc.dram_tensor("xp", [ROWS_PER_CORE, PADLEN], F16,
                        kind="ExternalInput").ap()
    zp = nc.dram_tensor("zp", [ROWS_PER_CORE, PADLEN], F16,
                        kind="ExternalInput").ap()
    wc = nc.dram_tensor("wc", [8, NI, 128, 128], F16, kind="ExternalInput").ap()
    ws = nc.dram_tensor("ws", [8, NI, 128, 128], F16, kind="ExternalInput").ap()
    wq = nc.dram_tensor("wq", [NI, 128, NBINS], F16, kind="ExternalInput").ap()
    pid = nc.dram_tensor("pid", [128, 128], F16, kind="ExternalInput").ap()
    out = nc.dram_tensor("out", [ROWS_PER_CORE, NBINS, NF], F32,
                         kind="ExternalOutput").ap()

    with tile.TileContext(nc) as tc:
        with ExitStack() as ctx:
            _emit(ctx, tc, xp, zp, wc, ws, wq, pid, out)
    nc.compile()
    return nc


def _emit(ctx, tc, xp, zp, wc, ws, wq, pid, out):
    nc = tc.nc
    SQ = mybir.ActivationFunctionType.Square
    SQRT = mybir.ActivationFunctionType.Sqrt
    LN = mybir.ActivationFunctionType.Ln

    consts = ctx.enter_context(tc.tile_pool(name="consts", bufs=1))
    natp = ctx.enter_context(tc.tile_pool(name="natp", bufs=2))
    stage = ctx.enter_context(tc.tile_pool(name="stage", bufs=2))
    eo = ctx.enter_context(tc.tile_pool(name="eo", bufs=2))
    magp = ctx.enter_context(tc.tile_pool(name="magp", bufs=2))
    sqp = ctx.enter_context(tc.tile_pool(name="sqp", bufs=4))
    outp = ctx.enter_context(tc.tile_pool(name="outp", bufs=2))
    ps_mm = ctx.enter_context(tc.tile_pool(name="ps_mm", bufs=3, space="PSUM"))
    ps_tp = ctx.enter_context(tc.tile_pool(name="ps_tp", bufs=4, space="PSUM"))
    ps_cq = ctx.enter_context(tc.tile_pool(name="ps_cq", bufs=1, space="PSUM"))

    # constants
    wc_sb = consts.tile([128, 8, NI, 128], F16, tag="wc_sb")
    ws_sb = consts.tile([128, 8, NI, 128], F16, tag="ws_sb")
    wq_sb = consts.tile([128, NI, NBINS], F16, tag="wq_sb")
    identp = consts.tile([128, 128], F16, tag="identp")
    nc.sync.dma_start(identp[:], pid)
    nc.gpsimd.dma_start(wc_sb[:], wc.rearrange("a i p f -> p a i f"))
    nc.scalar.dma_start(ws_sb[:], ws.rearrange("a i p f -> p a i f"))
    nc.sync.dma_start(wq_sb[:], wq.rearrange("i p k -> p i k"))
    lnbias = consts.tile([128, 1], F32, tag="lnbias")
    nc.gpsimd.memset(lnbias[:], 1e-10)

    def emit_stage(r, it):
        """DMA + permuted PE transpose + copyback + fold add/sub."""
        f0 = T_STARTS[it]
        cbase = 4 * f0
        natx = natp.tile([128, NGRP, 128], F16, tag="natx")
        natz = natp.tile([128, NGRP, 128], F16, tag="natz")
        # natx[c, g, s] = xpad[(cbase+128g+c)*128 + s + 1]
        nc.sync.dma_start(
            natx[:],
            xp[r, 128 * cbase + 1: 128 * (cbase + WCH) + 1].rearrange(
                "(g c s) -> c g s", g=NGRP, s=128),
        )
        nc.sync.dma_start(
            natz[:],
            zp[r, 128 * cbase: 128 * (cbase + WCH)].rearrange(
                "(g c s) -> c g s", g=NGRP, s=128),
        )
        dts = stage.tile([128, 4, Q], F16, tag="dts")
        rev = stage.tile([128, 4, Q], F16, tag="rev")
        for g in range(NGRP):
            tpx = ps_tp.tile([128, 128], F16, tag="tp")
            nc.tensor.transpose(tpx[:], natx[:, g], identp[:])
            # tpx[s, a*32+q] = chunk(4q+a) -> dts[s, a, 32g+q]
            nc.vector.tensor_copy(
                dts[:, :, 32 * g: 32 * (g + 1)],
                tpx.rearrange("p (a q) -> p a q", a=4))
            tpz = ps_tp.tile([128, 128], F16, tag="tp")
            nc.tensor.transpose(tpz[:], natz[:, g], identp[:])
            nc.vector.tensor_copy(
                rev[:, :, 32 * g: 32 * (g + 1)],
                tpz.rearrange("p (a q) -> p a q", a=4))

        # folded operands: E[j,t]=x[512t+j+1]+x[512t+2047-j], O = diff
        T = T_SIZES[it]
        e4 = eo.tile([128, 8, T_ALLOC], F16, tag="e4")
        o4 = eo.tile([128, 8, T_ALLOC], F16, tag="o4")
        for a in range(8):
            d_ap = dts[:, a % 4, a // 4: a // 4 + T]
            r_ap = rev[:, (15 - a) % 4, (15 - a) // 4: (15 - a) // 4 + T]
            nc.vector.tensor_add(e4[:, a, :T], d_ap, r_ap)
            sub_eng = nc.vector if a % 2 == 0 else nc.gpsimd
            sub_eng.tensor_sub(o4[:, a, :T], d_ap, r_ap)
        return e4, o4

    def emit_dft(r, it, e4, o4):
        """DFT matmuls + magnitude for one frame tile (512 bins)."""
        T = T_SIZES[it]
        mag = magp.tile([128, NI, T_ALLOC], F16, tag="mag")
        for i in range(NI):
            ps_re = ps_mm.tile([128, T_ALLOC], F32, tag="mm")
            for a in range(8):
                nc.tensor.matmul(
                    ps_re[:, :T], wc_sb[:, a, i], e4[:, a, :T],
                    start=(a == 0), stop=(a == 7),
                )
            ps_im = ps_mm.tile([128, T_ALLOC], F32, tag="mm")
            for a in range(8):
                nc.tensor.matmul(
                    ps_im[:, :T], ws_sb[:, a, i], o4[:, a, :T],
                    start=(a == 0), stop=(a == 7),
                )
            sq = sqp.tile([128, T_ALLOC], F32, tag="sq")
            nc.scalar.activation(sq[:, :T], ps_re[:, :T], SQ)
            sq2 = sqp.tile([128, T_ALLOC], F32, tag="sq2")
            nc.scalar.activation(sq2[:, :T], ps_im[:, :T], SQ)
            nc.gpsimd.tensor_add(sq[:, :T], sq[:, :T], sq2[:, :T])
            nc.scalar.activation(mag[:, i, :T], sq[:, :T], SQRT)
        return mag

    def emit_cqt(r, it, mag):
        """CQT projection, log10, store."""
        T = T_SIZES[it]
        f0 = T_STARTS[it]
        ps_c = ps_cq.tile([NBINS, T_ALLOC], F32, tag="ps_c")
        for i in range(NI):
            nc.tensor.matmul(
                ps_c[:, :T], wq_sb[:, i, :], mag[:, i, :T],
                start=(i == 0), stop=(i == NI - 1),
            )
        V = min(T, NF - f0)          # valid (non-garbage) frames
        outt = outp.tile([NBINS, T_ALLOC], F32, tag="outt")
        nc.scalar.activation(outt[:, :V], ps_c[:, :V], LN,
                             bias=lnbias[:NBINS])
        nc.gpsimd.tensor_scalar_mul(outt[:, :V], outt[:, :V], LOG10E)
        nc.sync.dma_start(out[r, :, f0: f0 + V], outt[:, :V])

    # software pipeline: PE order per slot is [transposes k+1][cqt k-1][dft k]
    tiles = [(r, it) for r in range(ROWS_PER_CORE) for it in range(6)]
    staged = emit_stage(*tiles[0])
    pending = None          # (r, it, mag) awaiting cqt
    for k, (r, it) in enumerate(tiles):
        nxt = emit_stage(*tiles[k + 1]) if k + 1 < len(tiles) else None
        if pending is not None:
            emit_cqt(*pending)
        mag = emit_dft(r, it, *staged)
        pending = (r, it, mag)
        staged = nxt
    emit_cqt(*pending)


_PROGRAM_CACHE = {}


def _get_program():
    if "nc" not in _PROGRAM_CACHE:
        _PROGRAM_CACHE["nc"] = _build_program()
    return _PROGRAM_CACHE["nc"]


def kernel(audio):
    audio = np.asarray(audio, dtype=np.float32)
    assert audio.shape == (B, L), audio.shape

    # host data movement: reflect pad + fp16 + within-chunk-reversed copy
    xpad = np.zeros((B, PADLEN), dtype=np.float16)
    xpad[:, :LP] = np.pad(audio, ((0, 0), (PAD, PAD)), mode="reflect")
    z = np.ascontiguousarray(
        xpad.reshape(B, NCH_PAD, 128)[:, :, ::-1]).reshape(B, PADLEN)

    wc, ws, wq = _host_tables()
    # (8_a, NI_i, 128_p, 128_f) blocks: wcb[a,i,p,f] = wc[128a+p, 128i+f]
    wc = np.ascontiguousarray(
        wc.reshape(8, 128, NI, 128).transpose(0, 2, 1, 3))
    ws = np.ascontiguousarray(
        ws.reshape(8, 128, NI, 128).transpose(0, 2, 1, 3))
    wq = np.ascontiguousarray(wq.reshape(NI, 128, NBINS))
    pid = _perm_identity()
    nc = _get_program()

    in_maps = []
    for c in range(NCORES):
        rows = slice(ROWS_PER_CORE * c, ROWS_PER_CORE * (c + 1))
        in_maps.append({
            "xp": np.ascontiguousarray(xpad[rows]),
            "zp": np.ascontiguousarray(z[rows]),
            "wc": wc, "ws": ws, "wq": wq, "pid": pid,
        })

    res = run_bass_kernel_spmd(nc, in_maps, core_ids=list(range(NCORES)))
    out = np.concatenate([res.results[c]["out"] for c in range(NCORES)], axis=0)
    return np.ascontiguousarray(out, dtype=np.float32)


# revision 12
# speedup vs baseline: 2.0313x; 1.3288x over previous
"""CQT extractor kernel for Trainium2 (8 NeuronCores, data-parallel over batch).

Pipeline per core (2 audio rows), all fp16 on the hot path:
  STFT-as-matmul with Hermitian folding (1024-long contraction), output
  truncated to the lowest 512 of 1025 rfft bins (CQT weights above ~5.5 kHz
  are < 2e-3 and contribute ~1e-4 relative error), magnitude via ACT
  Square/Sqrt split across scalar/vector/gpsimd, CQT projection matmul, log10.

The PE transposes use a permuted identity that de-interleaves the 4
chunks-per-hop phase structure, so every later access is unit-stride and the
PSUM->SBUF copies qualify for the DVE 2-byte fast path.
"""

import math
from contextlib import ExitStack

import numpy as np

import concourse.tile as tile
from concourse import bacc, mybir
from concourse.bass_utils import run_bass_kernel_spmd

# ---- problem constants (hardcoded per contest rules) ----
B = 16
L = 1310720
SR = 22050
HOP = 512
NFFT = 2048
NBINS = 84
BPO = 12
FMIN = 27.5

NF = 1 + L // HOP            # 2561 frames
PAD = NFFT // 2              # 1024
LP = L + 2 * PAD             # 1312768 reflect-padded length

NCORES = 8
ROWS_PER_CORE = B // NCORES  # 2

NFREQ = 512                  # truncated rfft bins (of 1025)
NI = NFREQ // 128            # 4 freq blocks

# frame tiling: 6 uniform tiles of 428 frames; frames past NF-1 are computed
# on zero padding and never written out
T_SIZES = [428] * 6
T_STARTS = [428 * i for i in range(6)]
T_ALLOC = 428

NGRP = 14                    # 128-chunk transpose groups per frame tile
WCH = NGRP * 128             # 1792 chunks staged per frame tile
Q = WCH // 4                 # 448 per-phase chunk columns
NCH_PAD = 4 * T_STARTS[-1] + WCH + 1   # chunks incl. zero pad (+1 for +1 shift)
PADLEN = 128 * NCH_PAD

F32 = mybir.dt.float32
F16 = mybir.dt.float16
LOG10E = 1.0 / math.log(10.0)


def _host_tables():
    """Folded DFT matrices (512 bins) and CQT weights, float64 -> float16."""
    j = np.arange(1024)
    n = (j + 1).astype(np.float64)          # contraction index j <-> sample n=j+1
    win = 0.5 * (1.0 - np.cos(2.0 * np.pi * n / NFFT))
    ang = 2.0 * np.pi * np.outer(n, np.arange(NFREQ, dtype=np.float64)) / NFFT
    wc = win[:, None] * np.cos(ang)
    ws = win[:, None] * np.sin(ang)
    wc[1023] *= 0.5           # n=1024 term is double-counted by the fold
    ws[1023] = 0.0
    sf = np.fft.rfftfreq(NFFT, 1.0 / SR)[:NFREQ]
    cf = FMIN * 2.0 ** (np.arange(NBINS, dtype=np.float64) / BPO)
    wq = np.exp(-np.abs(sf[:, None] - cf[None, :]) / (0.1 * cf[None, :]))
    return (
        np.ascontiguousarray(wc, dtype=np.float16),
        np.ascontiguousarray(ws, dtype=np.float16),
        np.ascontiguousarray(wq, dtype=np.float16),
    )


def _perm_identity():
    """Permutation matrices for the de-interleaving PE transposes.

    pid[0] (x stream): output column a*32+q <- chunk row 4q+a.
    pid[1] (z stream): output column b*32+q <- chunk row 4q+3-b, so the fold
    partner phases come out pre-flipped and the fold collapses to two wide
    unit-stride adds.
    """
    p = np.zeros((2, 128, 128), dtype=np.float16)
    jj = np.arange(128)
    p[0, 4 * (jj % 32) + jj // 32, jj] = 1.0
    p[1, 4 * (jj % 32) + 3 - jj // 32, jj] = 1.0
    return p


def _build_program():
    nc = bacc.Bacc("TRN2", target_bir_lowering=False, debug=False,
                   num_devices=NCORES)
    xp = nc.dram_tensor("xp", [ROWS_PER_CORE, PADLEN], F16,
                        kind="ExternalInput").ap()
    zp = nc.dram_tensor("zp", [ROWS_PER_CORE, PADLEN], F16,
                        kind="ExternalInput").ap()
    wc = nc.dram_tensor("wc", [8, NI, 128, 128], F16, kind="ExternalInput").ap()
    ws = nc.dram_tensor("ws", [8, NI, 128, 128], F16, kind="ExternalInput").ap()
    wq = nc.dram_tensor("wq", [NI, 128, NBINS], F16, kind="ExternalInput").ap()
    pid = nc.dram_tensor("pid", [2, 128, 128], F16, kind="ExternalInput").ap()
    out = nc.dram_tensor("out", [ROWS_PER_CORE, NBINS, NF], F32,
                         kind="ExternalOutput").ap()

    with tile.TileContext(nc) as tc:
        with ExitStack() as ctx:
            _emit(ctx, tc, xp, zp, wc, ws, wq, pid, out)
    nc.compile()
    return nc


def _emit(ctx, tc, xp, zp, wc, ws, wq, pid, out):
    nc = tc.nc
    SQ = mybir.ActivationFunctionType.Square
    SQRT = mybir.ActivationFunctionType.Sqrt
    LN = mybir.ActivationFunctionType.Ln

    consts = ctx.enter_context(tc.tile_pool(name="consts", bufs=1))
    natp = ctx.enter_context(tc.tile_pool(name="natp", bufs=2))
    stage = ctx.enter_context(tc.tile_pool(name="stage", bufs=2))
    eo = ctx.enter_context(tc.tile_pool(name="eo", bufs=2))
    magp = ctx.enter_context(tc.tile_pool(name="magp", bufs=2))
    sqp = ctx.enter_context(tc.tile_pool(name="sqp", bufs=2))
    outp = ctx.enter_context(tc.tile_pool(name="outp", bufs=2))
    ps_mm = ctx.enter_context(tc.tile_pool(name="ps_mm", bufs=3, space="PSUM"))
    ps_tp = ctx.enter_context(tc.tile_pool(name="ps_tp", bufs=4, space="PSUM"))
    ps_cq = ctx.enter_context(tc.tile_pool(name="ps_cq", bufs=1, space="PSUM"))

    # constants
    wc_sb = consts.tile([128, 8, NI, 128], F16, tag="wc_sb")
    ws_sb = consts.tile([128, 8, NI, 128], F16, tag="ws_sb")
    wq_sb = consts.tile([128, NI, NBINS], F16, tag="wq_sb")
    identx = consts.tile([128, 128], F16, tag="identx")
    identz = consts.tile([128, 128], F16, tag="identz")
    nc.sync.dma_start(identx[:], pid[0])
    nc.sync.dma_start(identz[:], pid[1])
    nc.gpsimd.dma_start(wc_sb[:], wc.rearrange("a i p f -> p a i f"))
    nc.scalar.dma_start(ws_sb[:], ws.rearrange("a i p f -> p a i f"))
    nc.sync.dma_start(wq_sb[:], wq.rearrange("i p k -> p i k"))
    lnbias = consts.tile([128, 1], F32, tag="lnbias")
    nc.gpsimd.memset(lnbias[:], 1e-10)

    def emit_stage(r, it):
        """DMA + permuted PE transpose + copyback + fold add/sub."""
        f0 = T_STARTS[it]
        cbase = 4 * f0
        natx = natp.tile([128, NGRP, 128], F16, tag="natx")
        natz = natp.tile([128, NGRP, 128], F16, tag="natz")
        # natx[c, g, s] = xpad[(cbase+128g+c)*128 + s + 1]
        nc.sync.dma_start(
            natx[:],
            xp[r, 128 * cbase + 1: 128 * (cbase + WCH) + 1].rearrange(
                "(g c s) -> c g s", g=NGRP, s=128),
        )
        nc.sync.dma_start(
            natz[:],
            zp[r, 128 * cbase: 128 * (cbase + WCH)].rearrange(
                "(g c s) -> c g s", g=NGRP, s=128),
        )
        dts = stage.tile([128, 4, Q], F16, tag="dts")
        rev = stage.tile([128, 4, Q], F16, tag="rev")
        for g in range(NGRP):
            tpx = ps_tp.tile([128, 128], F16, tag="tp")
            nc.tensor.transpose(tpx[:], natx[:, g], identx[:])
            # tpx[s, a*32+q] = chunk(4q+a) -> dts[s, a, 32g+q]
            nc.vector.tensor_copy(
                dts[:, :, 32 * g: 32 * (g + 1)],
                tpx.rearrange("p (a q) -> p a q", a=4))
            tpz = ps_tp.tile([128, 128], F16, tag="tp")
            nc.tensor.transpose(tpz[:], natz[:, g], identz[:])
            # tpz[s, b*32+q] = z-chunk(4q+3-b) -> rev[s, b, 32g+q]
            nc.vector.tensor_copy(
                rev[:, :, 32 * g: 32 * (g + 1)],
                tpz.rearrange("p (a q) -> p a q", a=4))

        # folded operands: E[j,t]=x[512t+j+1]+x[512t+2047-j], O = diff.
        # rev[p,b,qq] = z-chunk(4qq+3-b), so partner of e4[:,a] (chunk
        # 4t+15-a) sits at rev[:, a%4, 3-(a//4)+t] -- two wide adds cover
        # a=0..3 and a=4..7 with unit-stride operands.
        T = T_SIZES[it]
        e4 = eo.tile([128, 8, T_ALLOC], F16, tag="e4")
        o4 = eo.tile([128, 8, T_ALLOC], F16, tag="o4")
        nc.vector.tensor_add(e4[:, 0:4, :T], dts[:, :, 0:T], rev[:, :, 3:3 + T])
        nc.vector.tensor_add(e4[:, 4:8, :T], dts[:, :, 1:1 + T], rev[:, :, 2:2 + T])
        nc.gpsimd.tensor_sub(o4[:, 0:4, :T], dts[:, :, 0:T], rev[:, :, 3:3 + T])
        nc.gpsimd.tensor_sub(o4[:, 4:8, :T], dts[:, :, 1:1 + T], rev[:, :, 2:2 + T])
        return e4, o4

    def emit_dft(r, it, e4, o4):
        """DFT matmuls + magnitude for one frame tile (512 bins)."""
        T = T_SIZES[it]
        mag = magp.tile([128, NI, T_ALLOC], F16, tag="mag")
        sq = sqp.tile([128, NI, T_ALLOC], F32, tag="sq")
        sq2 = sqp.tile([128, NI, T_ALLOC], F32, tag="sq2")
        for i in range(NI):
            ps_re = ps_mm.tile([128, T_ALLOC], F32, tag="mm")
            for a in range(8):
                nc.tensor.matmul(
                    ps_re[:, :T], wc_sb[:, a, i], e4[:, a, :T],
                    start=(a == 0), stop=(a == 7),
                )
            ps_im = ps_mm.tile([128, T_ALLOC], F32, tag="mm")
            for a in range(8):
                nc.tensor.matmul(
                    ps_im[:, :T], ws_sb[:, a, i], o4[:, a, :T],
                    start=(a == 0), stop=(a == 7),
                )
            nc.scalar.activation(sq[:, i, :T], ps_re[:, :T], SQ)
            nc.scalar.activation(sq2[:, i, :T], ps_im[:, :T], SQ)
        # one wide |.|^2 sum + sqrt for the whole tile
        nc.vector.tensor_add(sq[:, :, :T], sq[:, :, :T], sq2[:, :, :T])
        nc.scalar.activation(mag[:, :, :T], sq[:, :, :T], SQRT)
        return mag

    def emit_cqt(r, it, mag):
        """CQT projection, log10, store."""
        T = T_SIZES[it]
        f0 = T_STARTS[it]
        ps_c = ps_cq.tile([NBINS, T_ALLOC], F32, tag="ps_c")
        for i in range(NI):
            nc.tensor.matmul(
                ps_c[:, :T], wq_sb[:, i, :], mag[:, i, :T],
                start=(i == 0), stop=(i == NI - 1),
            )
        V = min(T, NF - f0)          # valid (non-garbage) frames
        outt = outp.tile([NBINS, T_ALLOC], F32, tag="outt")
        nc.scalar.activation(outt[:, :V], ps_c[:, :V], LN,
                             bias=lnbias[:NBINS])
        nc.vector.tensor_scalar_mul(outt[:, :V], outt[:, :V], LOG10E)
        nc.sync.dma_start(out[r, :, f0: f0 + V], outt[:, :V])

    # software pipeline: PE order per slot is [transposes k+1][cqt k-1][dft k]
    tiles = [(r, it) for r in range(ROWS_PER_CORE) for it in range(6)]
    staged = emit_stage(*tiles[0])
    pending = None          # (r, it, mag) awaiting cqt
    for k, (r, it) in enumerate(tiles):
        nxt = emit_stage(*tiles[k + 1]) if k + 1 < len(tiles) else None
        if pending is not None:
            emit_cqt(*pending)
        mag = emit_dft(r, it, *staged)
        pending = (r, it, mag)
        staged = nxt
    emit_cqt(*pending)


_PROGRAM_CACHE = {}


def _get_program():
    if "nc" not in _PROGRAM_CACHE:
        _PROGRAM_CACHE["nc"] = _build_program()
    return _PROGRAM_CACHE["nc"]


def kernel(audio):
    audio = np.asarray(audio, dtype=np.float32)
    assert audio.shape == (B, L), audio.shape

    # host data movement: reflect pad + fp16 + within-chunk-reversed copy
    xpad = np.zeros((B, PADLEN), dtype=np.float16)
    xpad[:, :LP] = np.pad(audio, ((0, 0), (PAD, PAD)), mode="reflect")
    z = np.ascontiguousarray(
        xpad.reshape(B, NCH_PAD, 128)[:, :, ::-1]).reshape(B, PADLEN)

    wc, ws, wq = _host_tables()
    # (8_a, NI_i, 128_p, 128_f) blocks: wcb[a,i,p,f] = wc[128a+p, 128i+f]
    wc = np.ascontiguousarray(
        wc.reshape(8, 128, NI, 128).transpose(0, 2, 1, 3))
    ws = np.ascontiguousarray(
        ws.reshape(8, 128, NI, 128).transpose(0, 2, 1, 3))
    wq = np.ascontiguousarray(wq.reshape(NI, 128, NBINS))
    pid = _perm_identity()
    nc = _get_program()

    in_maps = []
    for c in range(NCORES):
        rows = slice(ROWS_PER_CORE * c, ROWS_PER_CORE * (c + 1))
        in_maps.append({
            "xp": np.ascontiguousarray(xpad[rows]),
            "zp": np.ascontiguousarray(z[rows]),
            "wc": wc, "ws": ws, "wq": wq, "pid": pid,
        })

    res = run_bass_kernel_spmd(nc, in_maps, core_ids=list(range(NCORES)))
    out = np.concatenate([res.results[c]["out"] for c in range(NCORES)], axis=0)
    return np.ascontiguousarray(out, dtype=np.float32)


# revision 14
# speedup vs baseline: 2.2036x; 1.0848x over previous
"""CQT extractor kernel for Trainium2 (8 NeuronCores, data-parallel over batch).

Pipeline per core (2 audio rows), all fp16 on the hot path:
  STFT-as-matmul with Hermitian folding (1024-long contraction), output
  truncated to the lowest 512 of 1025 rfft bins (CQT weights above ~5.5 kHz
  are < 2e-3 and contribute ~1e-4 relative error), magnitude via ACT
  Square/Sqrt split across scalar/vector/gpsimd, CQT projection matmul, log10.

The PE transposes use a permuted identity that de-interleaves the 4
chunks-per-hop phase structure, so every later access is unit-stride and the
PSUM->SBUF copies qualify for the DVE 2-byte fast path.
"""

import math
from contextlib import ExitStack

import numpy as np

import concourse.tile as tile
from concourse import bacc, mybir
from concourse.bass_utils import run_bass_kernel_spmd

# ---- problem constants (hardcoded per contest rules) ----
B = 16
L = 1310720
SR = 22050
HOP = 512
NFFT = 2048
NBINS = 84
BPO = 12
FMIN = 27.5

NF = 1 + L // HOP            # 2561 frames
PAD = NFFT // 2              # 1024
LP = L + 2 * PAD             # 1312768 reflect-padded length

NCORES = 8
ROWS_PER_CORE = B // NCORES  # 2

NFREQ = 512                  # truncated rfft bins (of 1025)
NI = NFREQ // 128            # 4 freq blocks

# frame tiling: 6 uniform tiles of 428 frames; frames past NF-1 are computed
# on zero padding and never written out
T_SIZES = [428] * 6
T_STARTS = [428 * i for i in range(6)]
T_ALLOC = 428

NGRP = 14                    # 128-chunk transpose groups per frame tile
WCH = NGRP * 128             # 1792 chunks staged per frame tile
Q = WCH // 4                 # 448 per-phase chunk columns
NCH_PAD = 4 * T_STARTS[-1] + WCH + 1   # chunks incl. zero pad (+1 for +1 shift)
PADLEN = 128 * NCH_PAD

F32 = mybir.dt.float32
F16 = mybir.dt.float16
LOG10E = 1.0 / math.log(10.0)


def _host_tables():
    """Folded DFT matrices (512 bins) and CQT weights, float64 -> float16."""
    j = np.arange(1024)
    n = (j + 1).astype(np.float64)          # contraction index j <-> sample n=j+1
    win = 0.5 * (1.0 - np.cos(2.0 * np.pi * n / NFFT))
    ang = 2.0 * np.pi * np.outer(n, np.arange(NFREQ, dtype=np.float64)) / NFFT
    wc = win[:, None] * np.cos(ang)
    ws = win[:, None] * np.sin(ang)
    wc[1023] *= 0.5           # n=1024 term is double-counted by the fold
    ws[1023] = 0.0
    sf = np.fft.rfftfreq(NFFT, 1.0 / SR)[:NFREQ]
    cf = FMIN * 2.0 ** (np.arange(NBINS, dtype=np.float64) / BPO)
    wq = np.exp(-np.abs(sf[:, None] - cf[None, :]) / (0.1 * cf[None, :]))
    return (
        np.ascontiguousarray(wc, dtype=np.float16),
        np.ascontiguousarray(ws, dtype=np.float16),
        np.ascontiguousarray(wq, dtype=np.float16),
    )


def _perm_identity():
    """Permutation matrices for the de-interleaving PE transposes.

    pid[0] (x stream): output column a*32+q <- chunk row 4q+a.
    pid[1] (z stream): output column b*32+q <- chunk row 4q+3-b, so the fold
    partner phases come out pre-flipped and the fold collapses to two wide
    unit-stride adds.
    """
    p = np.zeros((2, 128, 128), dtype=np.float16)
    jj = np.arange(128)
    p[0, 4 * (jj % 32) + jj // 32, jj] = 1.0
    p[1, 4 * (jj % 32) + 3 - jj // 32, jj] = 1.0
    return p


def _build_program():
    nc = bacc.Bacc("TRN2", target_bir_lowering=False, debug=False,
                   num_devices=NCORES)
    xp = nc.dram_tensor("xp", [ROWS_PER_CORE, PADLEN], F16,
                        kind="ExternalInput").ap()
    zp = nc.dram_tensor("zp", [ROWS_PER_CORE, PADLEN], F16,
                        kind="ExternalInput").ap()
    wc = nc.dram_tensor("wc", [8, NI, 128, 128], F16, kind="ExternalInput").ap()
    ws = nc.dram_tensor("ws", [8, NI, 128, 128], F16, kind="ExternalInput").ap()
    wq = nc.dram_tensor("wq", [NI, 128, NBINS], F16, kind="ExternalInput").ap()
    pid = nc.dram_tensor("pid", [2, 128, 128], F16, kind="ExternalInput").ap()
    out = nc.dram_tensor("out", [ROWS_PER_CORE, NBINS, NF], F32,
                         kind="ExternalOutput").ap()

    with tile.TileContext(nc) as tc:
        with ExitStack() as ctx:
            _emit(ctx, tc, xp, zp, wc, ws, wq, pid, out)
    nc.compile()
    return nc


def _emit(ctx, tc, xp, zp, wc, ws, wq, pid, out):
    nc = tc.nc
    SQ = mybir.ActivationFunctionType.Square
    SQRT = mybir.ActivationFunctionType.Sqrt
    LN = mybir.ActivationFunctionType.Ln

    consts = ctx.enter_context(tc.tile_pool(name="consts", bufs=1))
    natp = ctx.enter_context(tc.tile_pool(name="natp", bufs=2))
    stage = ctx.enter_context(tc.tile_pool(name="stage", bufs=2))
    eo = ctx.enter_context(tc.tile_pool(name="eo", bufs=2))
    magp = ctx.enter_context(tc.tile_pool(name="magp", bufs=2))
    sqp = ctx.enter_context(tc.tile_pool(name="sqp", bufs=2))
    outp = ctx.enter_context(tc.tile_pool(name="outp", bufs=2))
    ps_mm = ctx.enter_context(tc.tile_pool(name="ps_mm", bufs=3, space="PSUM"))
    ps_tp = ctx.enter_context(tc.tile_pool(name="ps_tp", bufs=4, space="PSUM"))
    ps_cq = ctx.enter_context(tc.tile_pool(name="ps_cq", bufs=1, space="PSUM"))

    # constants
    wc_sb = consts.tile([128, 8, NI, 128], F16, tag="wc_sb")
    ws_sb = consts.tile([128, 8, NI, 128], F16, tag="ws_sb")
    wq_sb = consts.tile([128, NI, NBINS], F16, tag="wq_sb")
    identx = consts.tile([128, 128], F16, tag="identx")
    identz = consts.tile([128, 128], F16, tag="identz")
    nc.sync.dma_start(identx[:], pid[0])
    nc.sync.dma_start(identz[:], pid[1])
    nc.gpsimd.dma_start(wc_sb[:], wc.rearrange("a i p f -> p a i f"))
    nc.scalar.dma_start(ws_sb[:], ws.rearrange("a i p f -> p a i f"))
    nc.sync.dma_start(wq_sb[:], wq.rearrange("i p k -> p i k"))
    lnbias = consts.tile([128, 1], F32, tag="lnbias")
    nc.gpsimd.memset(lnbias[:], 1e-10)

    def emit_stage(r, it):
        """DMA + permuted PE transpose + copyback + fold add/sub."""
        f0 = T_STARTS[it]
        cbase = 4 * f0
        natx = natp.tile([128, NGRP, 128], F16, tag="natx")
        natz = natp.tile([128, NGRP, 128], F16, tag="natz")
        # natx[c, g, s] = xpad[(cbase+128g+c)*128 + s + 1]
        nc.sync.dma_start(
            natx[:],
            xp[r, 128 * cbase + 1: 128 * (cbase + WCH) + 1].rearrange(
                "(g c s) -> c g s", g=NGRP, s=128),
        )
        nc.sync.dma_start(
            natz[:],
            zp[r, 128 * cbase: 128 * (cbase + WCH)].rearrange(
                "(g c s) -> c g s", g=NGRP, s=128),
        )
        dts = stage.tile([128, 4, Q], F16, tag="dts")
        rev = stage.tile([128, 4, Q], F16, tag="rev")
        # 7 transposes share one PSUM bank (fp16 7*128*2B = 1792B/partition)
        # as one accumulation group over disjoint columns, then one wide copy
        for half in range(2):
            for nat, ident, dst in ((natx, identx, dts), (natz, identz, rev)):
                pack = ps_tp.tile([128, 7, 128], F16, tag="tp")
                for g7 in range(7):
                    g = 7 * half + g7
                    nc.tensor.matmul(
                        pack[:, g7], nat[:, g], ident[:],
                        is_transpose=True, start=(g7 == 0), stop=(g7 == 6),
                    )
                # pack[s, g7, a*32+q] = chunk(4q+phi) -> dst[s, a, 224h+32g7+q]
                nc.vector.tensor_copy(
                    dst[:, :, 224 * half: 224 * (half + 1)].rearrange(
                        "p a (g q) -> p a g q", g=7),
                    pack.rearrange("p g (a q) -> p a g q", a=4))

        # folded operands: E[j,t]=x[512t+j+1]+x[512t+2047-j], O = diff.
        # rev[p,b,qq] = z-chunk(4qq+3-b), so partner of e4[:,a] (chunk
        # 4t+15-a) sits at rev[:, a%4, 3-(a//4)+t] -- two wide adds cover
        # a=0..3 and a=4..7 with unit-stride operands.
        T = T_SIZES[it]
        e4 = eo.tile([128, 8, T_ALLOC], F16, tag="e4")
        o4 = eo.tile([128, 8, T_ALLOC], F16, tag="o4")
        nc.vector.tensor_add(e4[:, 0:4, :T], dts[:, :, 0:T], rev[:, :, 3:3 + T])
        nc.vector.tensor_add(e4[:, 4:8, :T], dts[:, :, 1:1 + T], rev[:, :, 2:2 + T])
        nc.gpsimd.tensor_sub(o4[:, 0:4, :T], dts[:, :, 0:T], rev[:, :, 3:3 + T])
        nc.gpsimd.tensor_sub(o4[:, 4:8, :T], dts[:, :, 1:1 + T], rev[:, :, 2:2 + T])
        return e4, o4

    def emit_dft(r, it, e4, o4):
        """DFT matmuls + magnitude for one frame tile (512 bins)."""
        T = T_SIZES[it]
        mag = magp.tile([128, NI, T_ALLOC], F16, tag="mag")
        sq = sqp.tile([128, NI, T_ALLOC], F32, tag="sq")
        sq2 = sqp.tile([128, NI, T_ALLOC], F32, tag="sq2")
        for i in range(NI):
            ps_re = ps_mm.tile([128, T_ALLOC], F32, tag="mm")
            for a in range(8):
                nc.tensor.matmul(
                    ps_re[:, :T], wc_sb[:, a, i], e4[:, a, :T],
                    start=(a == 0), stop=(a == 7),
                )
            ps_im = ps_mm.tile([128, T_ALLOC], F32, tag="mm")
            for a in range(8):
                nc.tensor.matmul(
                    ps_im[:, :T], ws_sb[:, a, i], o4[:, a, :T],
                    start=(a == 0), stop=(a == 7),
                )
            nc.scalar.activation(sq[:, i, :T], ps_re[:, :T], SQ)
            nc.scalar.activation(sq2[:, i, :T], ps_im[:, :T], SQ)
        # one wide |.|^2 sum + sqrt for the whole tile
        nc.vector.tensor_add(sq[:, :, :T], sq[:, :, :T], sq2[:, :, :T])
        nc.scalar.activation(mag[:, :, :T], sq[:, :, :T], SQRT)
        return mag

    def emit_cqt(r, it, mag):
        """CQT projection, log10, store."""
        T = T_SIZES[it]
        f0 = T_STARTS[it]
        ps_c = ps_cq.tile([NBINS, T_ALLOC], F32, tag="ps_c")
        for i in range(NI):
            nc.tensor.matmul(
                ps_c[:, :T], wq_sb[:, i, :], mag[:, i, :T],
                start=(i == 0), stop=(i == NI - 1),
            )
        V = min(T, NF - f0)          # valid (non-garbage) frames
        outt = outp.tile([NBINS, T_ALLOC], F32, tag="outt")
        nc.scalar.activation(outt[:, :V], ps_c[:, :V], LN,
                             bias=lnbias[:NBINS])
        nc.scalar.mul(outt[:, :V], outt[:, :V], LOG10E)
        nc.sync.dma_start(out[r, :, f0: f0 + V], outt[:, :V])

    # software pipeline: PE order per slot is [transposes k+1][cqt k-1][dft k]
    tiles = [(r, it) for r in range(ROWS_PER_CORE) for it in range(6)]
    staged = emit_stage(*tiles[0])
    pending = None          # (r, it, mag) awaiting cqt
    for k, (r, it) in enumerate(tiles):
        nxt = emit_stage(*tiles[k + 1]) if k + 1 < len(tiles) else None
        if pending is not None:
            emit_cqt(*pending)
        mag = emit_dft(r, it, *staged)
        pending = (r, it, mag)
        staged = nxt
    emit_cqt(*pending)


_PROGRAM_CACHE = {}


def _get_program():
    if "nc" not in _PROGRAM_CACHE:
        _PROGRAM_CACHE["nc"] = _build_program()
    return _PROGRAM_CACHE["nc"]


def kernel(audio):
    audio = np.asarray(audio, dtype=np.float32)
    assert audio.shape == (B, L), audio.shape

    # host data movement: reflect pad + fp16 + within-chunk-reversed copy
    xpad = np.zeros((B, PADLEN), dtype=np.float16)
    xpad[:, :LP] = np.pad(audio, ((0, 0), (PAD, PAD)), mode="reflect")
    z = np.ascontiguousarray(
        xpad.reshape(B, NCH_PAD, 128)[:, :, ::-1]).reshape(B, PADLEN)

    wc, ws, wq = _host_tables()
    # (8_a, NI_i, 128_p, 128_f) blocks: wcb[a,i,p,f] = wc[128a+p, 128i+f]
    wc = np.ascontiguousarray(
        wc.reshape(8, 128, NI, 128).transpose(0, 2, 1, 3))
    ws = np.ascontiguousarray(
        ws.reshape(8, 128, NI, 128).transpose(0, 2, 1, 3))
    wq = np.ascontiguousarray(wq.reshape(NI, 128, NBINS))
    pid = _perm_identity()
    nc = _get_program()

    in_maps = []
    for c in range(NCORES):
        rows = slice(ROWS_PER_CORE * c, ROWS_PER_CORE * (c + 1))
        in_maps.append({
            "xp": np.ascontiguousarray(xpad[rows]),
            "zp": np.ascontiguousarray(z[rows]),
            "wc": wc, "ws": ws, "wq": wq, "pid": pid,
        })

    res = run_bass_kernel_spmd(nc, in_maps, core_ids=list(range(NCORES)))
    out = np.concatenate([res.results[c]["out"] for c in range(NCORES)], axis=0)
    return np.ascontiguousarray(out, dtype=np.float32)
